# revision 23
# baseline (speedup 1.0000x reference)
"""ASPP + pixel-shuffle upsample + 1x1 project, on 8 TRN2 NeuronCores.

Strategy: data-parallel over batch (B=8 -> 1 image per core). Per core:
  - all convs as matmuls on the PE (bf16 inputs/weights, fp32 PSUM accum)
  - BN folded into conv weights/bias on host
  - 3x3 dilated convs = 9 shifted 1x1 taps accumulated in PSUM; each tap
    computes only its valid (non-zero-padding) region. PSUM spatial chunks
    are laid out column-major so a tap's column restriction is a contiguous
    PSUM range; x is stored row-major with 18 zero rows of top/bottom pad
    (row-shifted taps read zero rows; fully-zero chunks are skipped).
  - interleave (pixel-shuffle) is never materialized: the 1x1 projection is
    applied per-branch and its ReLU output is written with a strided AP
    directly into the interleaved position of the output row buffer
  - output rows stream back to DRAM per 16-row block
"""

import numpy as np
import ml_dtypes

B, CIN, COUT, H = 8, 256, 128, 64
PAD = 18
XR = H + 2 * PAD  # padded rows: 100
EPS = 1e-5
RATES = (6, 12, 18)
N_CORES = 8
NTAP = 28  # 1 (branch0 1x1) + 3 branches * 9 taps

_BF16 = ml_dtypes.bfloat16


def _branch_taps(t):
    """[(weight_block, sy, sx)] for branch t, center tap first."""
    if t == 0:
        return [(0, 0, 0)]
    d = RATES[t - 1]
    base = 1 + 9 * (t - 1)
    taps = []
    for ky in range(3):
        for kx in range(3):
            taps.append((base + ky * 3 + kx, (ky - 1) * d, (kx - 1) * d))
    taps.sort(key=lambda w: (w[1] != 0 or w[2] != 0))  # center first
    return taps


def build_program():
    import concourse.mybir as mybir
    import concourse.tile as tile
    from concourse import bacc

    f32, bf16 = mybir.dt.float32, mybir.dt.bfloat16
    Relu = mybir.ActivationFunctionType.Relu

    nc = bacc.Bacc("TRN2", target_bir_lowering=False, debug=False)
    xp = nc.dram_tensor("xp", [2, 128, H * XR], bf16, kind="ExternalInput")
    wb = nc.dram_tensor("wb", [2, 128, NTAP * 128], bf16, kind="ExternalInput")
    wp = nc.dram_tensor("wp", [128, 128], bf16, kind="ExternalInput")
    bias = nc.dram_tensor("bias", [128, 5], f32, kind="ExternalInput")
    out = nc.dram_tensor("out", [128, 4 * H * H], f32, kind="ExternalOutput")

    with tile.TileContext(nc) as tc:
        with (
            tc.tile_pool(name="const", bufs=1) as cpool,
            tc.tile_pool(name="bf", bufs=3) as bfpool,
            tc.tile_pool(name="ob", bufs=3) as opool,
            tc.tile_pool(name="psA", bufs=3, space="PSUM") as psA,
            tc.tile_pool(name="psB", bufs=3, space="PSUM") as psB,
        ):
            # PE warm-up: dummy matmuls on a zeroed scratch tile release the
            # HAM clock throttle while the input DMAs are still in flight
            scratch = cpool.tile([128, 512], bf16, tag="scratch")
            nc.vector.memset(scratch[:], 0.0)
            psW = psA.tile([128, 512], f32, tag="warm", bufs=1)
            for i in range(20):
                nc.tensor.matmul(
                    psW[:], lhsT=scratch[:, :128], rhs=scratch[:],
                    start=(i == 0), stop=(i == 19), skip_group_check=True,
                )
            bt = cpool.tile([128, 5], f32, tag="bias")
            nc.sync.dma_start(out=bt, in_=bias[:])
            wpt = cpool.tile([128, 128], bf16, tag="wp")
            nc.sync.dma_start(out=wpt, in_=wp[:])
            # x stored column-major: [128, 64 cols x 100 rows], rows 18..82
            # hold the image (transposed + row-padded on host), so the DMA is
            # fully contiguous and matmul rhs APs have 8 contiguous rows
            # innermost. Issue order: x chunk 0, then the weights the first
            # chunk's branches need, then x chunk 1, then branch-3 weights.
            wt = [
                cpool.tile([128, NTAP * 128], bf16, tag=f"w{c}", name=f"w{c}")
                for c in range(2)
            ]
            xtile = [
                cpool.tile([128, H * XR], bf16, tag=f"x{c}", name=f"x{c}")
                for c in range(2)
            ]
            x3t = [
                xtile[c].rearrange("p (w h) -> p w h", h=XR) for c in range(2)
            ]
            nc.sync.dma_start(out=xtile[0], in_=xp[0])
            nc.sync.dma_start(out=wt[0][:, : 19 * 128], in_=wb[0][:, : 19 * 128])
            nc.sync.dma_start(out=xtile[1], in_=xp[1])
            nc.sync.dma_start(out=wt[1][:, : 19 * 128], in_=wb[1][:, : 19 * 128])
            for c in range(2):  # branch 3 weights last
                nc.sync.dma_start(out=wt[c][:, 19 * 128 :], in_=wb[c][:, 19 * 128 :])

            out3 = out.rearrange("p (a b) -> p a b", b=2 * H)
            for k in range(8):  # 8-row input chunks -> output rows 16k..16k+16
                ob = opool.tile([128, 16 * 2 * H], f32, tag="ob")
                ob3 = ob.rearrange("p (a b) -> p a b", b=2 * H)
                # (out-col, out-row) view matching the col-major psum layout
                obt = ob3.rearrange("p a b -> p b a")
                done = set()
                # k=0: branches with long cin-chunk-0 prefixes first, so the
                # PE has work before x chunk 1 lands
                for t in ([1, 2, 3, 0] if k == 0 else range(4)):
                    ps = psA.tile([128, 512], f32, tag="ps")
                    mms = []
                    for c in range(2):  # all cin-chunk-0 taps first
                        for blk, sy, sx in _branch_taps(t):
                            if 8 * k + 8 + sy <= 0 or 8 * k + sy >= H:
                                continue  # all rows read zero-pad: contributes 0
                            c0, c1 = max(0, -sx), min(H, H - sx)
                            mms.append((blk, sy, sx, c0, c1, c))
                    n = len(mms)
                    for idx, (blk, sy, sx, c0, c1, c) in enumerate(mms):
                        r0 = PAD + 8 * k + sy
                        rhs = x3t[c][:, c0 + sx : c1 + sx, r0 : r0 + 8]
                        dst = ps[:] if (c1 - c0) == H else ps[:, c0 * 8 : c1 * 8]
                        nc.tensor.matmul(
                            dst,
                            lhsT=wt[c][:, blk * 128 : (blk + 1) * 128],
                            rhs=rhs,
                            start=(idx == 0),
                            stop=(idx == n - 1),
                        )
                    bftile = bfpool.tile([128, 512], bf16, tag="bf")
                    nc.scalar.activation(bftile[:], ps[:], Relu, bias=bt[:, t : t + 1])
                    ps2 = psB.tile([128, 512], f32, tag="ps2")
                    nc.tensor.matmul(
                        ps2[:], lhsT=wpt[:], rhs=bftile[:], start=True, stop=True
                    )
                    r_, c_ = t // 2, t % 2
                    nc.scalar.activation(
                        obt[:, c_::2, r_::2],
                        ps2.rearrange("p (w h) -> p w h", h=8),
                        Relu,
                        bias=bt[:, 4:5],
                    )
                    done.add(t)
                    # stream each output-row parity out as soon as the two
                    # branches feeding it are done
                    if done >= {0, 1} and "even" not in done:
                        nc.sync.dma_start(
                            out=out3[:, 16 * k : 16 * (k + 1) : 2, :],
                            in_=ob3[:, 0::2, :],
                        )
                        done.add("even")
                    if done >= {2, 3} and "odd" not in done:
                        nc.sync.dma_start(
                            out=out3[:, 16 * k + 1 : 16 * (k + 1) : 2, :],
                            in_=ob3[:, 1::2, :],
                        )
                        done.add("odd")
    nc.compile()
    return nc


def host_prep_weights(inputs):
    f32 = np.float32
    scales, biases = [], []
    for t in ("0", "1", "2", "3", "p"):
        g = np.asarray(inputs[f"g{t}"], f32)
        b = np.asarray(inputs[f"b{t}"], f32)
        m = np.asarray(inputs[f"m{t}"], f32)
        v = np.asarray(inputs[f"v{t}"], f32)
        s = g / np.sqrt(v + EPS)
        scales.append(s)
        biases.append((b - m * s).astype(f32))
    bias_arr = np.stack(biases, axis=1).astype(f32)  # (128, 5)

    wtaps = np.zeros((NTAP, CIN, COUT), f32)  # [tap, ci, co]
    w0 = np.asarray(inputs["w0"], f32)[:, :, 0, 0] * scales[0][:, None]  # (co, ci)
    wtaps[0] = w0.T
    blk = 1
    for bi, key in enumerate(("w1", "w2", "w3")):
        w = np.asarray(inputs[key], f32) * scales[bi + 1][:, None, None, None]
        for ky in range(3):
            for kx in range(3):
                wtaps[blk] = w[:, :, ky, kx].T
                blk += 1
    wb = (
        wtaps.reshape(NTAP, 2, 128, COUT)
        .transpose(1, 2, 0, 3)
        .reshape(2, 128, NTAP * COUT)
        .astype(_BF16)
    )
    wpT = (
        (np.asarray(inputs["wp"], f32)[:, :, 0, 0] * scales[4][:, None])
        .T.astype(_BF16)
        .copy()
    )
    return wb, wpT, bias_arr


def host_prep_x(x):
    # transpose each image to (col, row) and bake the 18-row top/bottom zero
    # pad, matching the device's column-major SBUF layout exactly, so the
    # device DMA is one contiguous copy per cin-chunk
    x = np.asarray(x, np.float32).reshape(B, 2, 128, H, H)
    xt = np.zeros((B, 2, 128, H, XR), np.float32)
    xt[:, :, :, :, PAD : PAD + H] = x.transpose(0, 1, 2, 4, 3)
    return xt.reshape(B, 2, 128, H * XR).astype(_BF16)


def make_in_maps(inputs):
    wb, wpT, bias_arr = host_prep_weights(inputs)
    xq = host_prep_x(inputs["x"])
    return [{"xp": xq[b], "wb": wb, "wp": wpT, "bias": bias_arr} for b in range(B)]


_NC_CACHE = []


def kernel(**inputs):
    from concourse import bass_utils

    if not _NC_CACHE:
        _NC_CACHE.append(build_program())
    nc = _NC_CACHE[0]
    in_maps = make_in_maps(inputs)
    res = bass_utils.run_bass_kernel_spmd(nc, in_maps, core_ids=list(range(N_CORES)))
    return np.stack(
        [r["out"].reshape(COUT, 2 * H, 2 * H) for r in res.results]
    ).astype(np.float32)


# revision 24
# speedup vs baseline: 1.0248x; 1.0248x over previous
"""ASPP + pixel-shuffle upsample + 1x1 project, on 8 TRN2 NeuronCores.

Strategy: data-parallel over batch (B=8 -> 1 image per core). Per core:
  - all convs as matmuls on the PE (bf16 inputs/weights, fp32 PSUM accum)
  - BN folded into conv weights/bias on host
  - 3x3 dilated convs = 9 shifted 1x1 taps accumulated in PSUM; each tap
    computes only its valid (non-zero-padding) region. PSUM spatial chunks
    are laid out column-major so a tap's column restriction is a contiguous
    PSUM range; x is stored row-major with 18 zero rows of top/bottom pad
    (row-shifted taps read zero rows; fully-zero chunks are skipped).
  - interleave (pixel-shuffle) is never materialized: the 1x1 projection is
    applied per-branch and its ReLU output is written with a strided AP
    directly into the interleaved position of the output row buffer
  - output rows stream back to DRAM per 16-row block
"""

import numpy as np
import ml_dtypes

B, CIN, COUT, H = 8, 256, 128, 64
PAD = 18
XR = H + 2 * PAD  # padded rows: 100
EPS = 1e-5
RATES = (6, 12, 18)
N_CORES = 8
NTAP = 28  # 1 (branch0 1x1) + 3 branches * 9 taps

_BF16 = ml_dtypes.bfloat16


def _branch_taps(t):
    """[(weight_block, sy, sx)] for branch t, center tap first."""
    if t == 0:
        return [(0, 0, 0)]
    d = RATES[t - 1]
    base = 1 + 9 * (t - 1)
    taps = []
    for ky in range(3):
        for kx in range(3):
            taps.append((base + ky * 3 + kx, (ky - 1) * d, (kx - 1) * d))
    taps.sort(key=lambda w: (w[1] != 0 or w[2] != 0))  # center first
    return taps


def build_program():
    import concourse.mybir as mybir
    import concourse.tile as tile
    from concourse import bacc

    f32, bf16 = mybir.dt.float32, mybir.dt.bfloat16
    Relu = mybir.ActivationFunctionType.Relu

    nc = bacc.Bacc("TRN2", target_bir_lowering=False, debug=False)
    xp = nc.dram_tensor("xp", [2, 128, H * XR], bf16, kind="ExternalInput")
    wb = nc.dram_tensor("wb", [2, 128, NTAP * 128], bf16, kind="ExternalInput")
    wp = nc.dram_tensor("wp", [128, 128], bf16, kind="ExternalInput")
    bias = nc.dram_tensor("bias", [128, 5], f32, kind="ExternalInput")
    out = nc.dram_tensor("out", [128, 4 * H * H], f32, kind="ExternalOutput")

    with tile.TileContext(nc) as tc:
        with (
            tc.tile_pool(name="const", bufs=1) as cpool,
            tc.tile_pool(name="bf", bufs=3) as bfpool,
            tc.tile_pool(name="ob", bufs=3) as opool,
            tc.tile_pool(name="psA", bufs=3, space="PSUM") as psA,
            tc.tile_pool(name="psB", bufs=3, space="PSUM") as psB,
        ):
            # PE warm-up: dummy matmuls on a zeroed scratch tile release the
            # HAM clock throttle while the input DMAs are still in flight
            scratch = cpool.tile([128, 512], bf16, tag="scratch")
            nc.vector.memset(scratch[:], 0.0)
            psW = psA.tile([128, 512], f32, tag="warm", bufs=1)
            for i in range(9):
                nc.tensor.matmul(
                    psW[:], lhsT=scratch[:, :128], rhs=scratch[:],
                    start=(i == 0), stop=(i == 8), skip_group_check=True,
                )
            bt = cpool.tile([128, 5], f32, tag="bias")
            nc.sync.dma_start(out=bt, in_=bias[:])
            wpt = cpool.tile([128, 128], bf16, tag="wp")
            nc.sync.dma_start(out=wpt, in_=wp[:])
            # x stored column-major: [128, 64 cols x 100 rows], rows 18..82
            # hold the image (transposed + row-padded on host), so the DMA is
            # fully contiguous and matmul rhs APs have 8 contiguous rows
            # innermost. Issue order: x chunk 0, then the weights the first
            # chunk's branches need, then x chunk 1, then branch-3 weights.
            wt = [
                cpool.tile([128, NTAP * 128], bf16, tag=f"w{c}", name=f"w{c}")
                for c in range(2)
            ]
            xtile = [
                cpool.tile([128, H * XR], bf16, tag=f"x{c}", name=f"x{c}")
                for c in range(2)
            ]
            x3t = [
                xtile[c].rearrange("p (w h) -> p w h", h=XR) for c in range(2)
            ]
            nc.sync.dma_start(out=xtile[0], in_=xp[0])
            nc.sync.dma_start(out=wt[0][:, : 19 * 128], in_=wb[0][:, : 19 * 128])
            nc.sync.dma_start(out=xtile[1], in_=xp[1])
            nc.sync.dma_start(out=wt[1][:, : 19 * 128], in_=wb[1][:, : 19 * 128])
            for c in range(2):  # branch 3 weights last
                nc.sync.dma_start(out=wt[c][:, 19 * 128 :], in_=wb[c][:, 19 * 128 :])

            out3 = out.rearrange("p (a b) -> p a b", b=2 * H)
            for k in range(8):  # 8-row input chunks -> output rows 16k..16k+16
                ob = opool.tile([128, 16 * 2 * H], f32, tag="ob")
                ob3 = ob.rearrange("p (a b) -> p a b", b=2 * H)
                # (out-col, out-row) view matching the col-major psum layout
                obt = ob3.rearrange("p a b -> p b a")
                done = set()
                # k=0: branches with long cin-chunk-0 prefixes first, so the
                # PE has work before x chunk 1 lands
                for t in ([1, 2, 3, 0] if k == 0 else range(4)):
                    ps = psA.tile([128, 512], f32, tag="ps")
                    mms = []
                    for c in range(2):  # all cin-chunk-0 taps first
                        for blk, sy, sx in _branch_taps(t):
                            if 8 * k + 8 + sy <= 0 or 8 * k + sy >= H:
                                continue  # all rows read zero-pad: contributes 0
                            c0, c1 = max(0, -sx), min(H, H - sx)
                            mms.append((blk, sy, sx, c0, c1, c))
                    n = len(mms)
                    for idx, (blk, sy, sx, c0, c1, c) in enumerate(mms):
                        r0 = PAD + 8 * k + sy
                        rhs = x3t[c][:, c0 + sx : c1 + sx, r0 : r0 + 8]
                        dst = ps[:] if (c1 - c0) == H else ps[:, c0 * 8 : c1 * 8]
                        nc.tensor.matmul(
                            dst,
                            lhsT=wt[c][:, blk * 128 : (blk + 1) * 128],
                            rhs=rhs,
                            start=(idx == 0),
                            stop=(idx == n - 1),
                        )
                    bftile = bfpool.tile([128, 512], bf16, tag="bf")
                    nc.scalar.activation(bftile[:], ps[:], Relu, bias=bt[:, t : t + 1])
                    ps2 = psB.tile([128, 512], f32, tag="ps2")
                    nc.tensor.matmul(
                        ps2[:], lhsT=wpt[:], rhs=bftile[:], start=True, stop=True
                    )
                    r_, c_ = t // 2, t % 2
                    nc.scalar.activation(
                        obt[:, c_::2, r_::2],
                        ps2.rearrange("p (w h) -> p w h", h=8),
                        Relu,
                        bias=bt[:, 4:5],
                    )
                    done.add(t)
                    # stream each output-row parity out as soon as the two
                    # branches feeding it are done
                    if done >= {0, 1} and "even" not in done:
                        nc.sync.dma_start(
                            out=out3[:, 16 * k : 16 * (k + 1) : 2, :],
                            in_=ob3[:, 0::2, :],
                        )
                        done.add("even")
                    if done >= {2, 3} and "odd" not in done:
                        nc.sync.dma_start(
                            out=out3[:, 16 * k + 1 : 16 * (k + 1) : 2, :],
                            in_=ob3[:, 1::2, :],
                        )
                        done.add("odd")
    nc.compile()
    return nc


def host_prep_weights(inputs):
    f32 = np.float32
    scales, biases = [], []
    for t in ("0", "1", "2", "3", "p"):
        g = np.asarray(inputs[f"g{t}"], f32)
        b = np.asarray(inputs[f"b{t}"], f32)
        m = np.asarray(inputs[f"m{t}"], f32)
        v = np.asarray(inputs[f"v{t}"], f32)
        s = g / np.sqrt(v + EPS)
        scales.append(s)
        biases.append((b - m * s).astype(f32))
    bias_arr = np.stack(biases, axis=1).astype(f32)  # (128, 5)

    wtaps = np.zeros((NTAP, CIN, COUT), f32)  # [tap, ci, co]
    w0 = np.asarray(inputs["w0"], f32)[:, :, 0, 0] * scales[0][:, None]  # (co, ci)
    wtaps[0] = w0.T
    blk = 1
    for bi, key in enumerate(("w1", "w2", "w3")):
        w = np.asarray(inputs[key], f32) * scales[bi + 1][:, None, None, None]
        for ky in range(3):
            for kx in range(3):
                wtaps[blk] = w[:, :, ky, kx].T
                blk += 1
    wb = (
        wtaps.reshape(NTAP, 2, 128, COUT)
        .transpose(1, 2, 0, 3)
        .reshape(2, 128, NTAP * COUT)
        .astype(_BF16)
    )
    wpT = (
        (np.asarray(inputs["wp"], f32)[:, :, 0, 0] * scales[4][:, None])
        .T.astype(_BF16)
        .copy()
    )
    return wb, wpT, bias_arr


def host_prep_x(x):
    # transpose each image to (col, row) and bake the 18-row top/bottom zero
    # pad, matching the device's column-major SBUF layout exactly, so the
    # device DMA is one contiguous copy per cin-chunk
    x = np.asarray(x, np.float32).reshape(B, 2, 128, H, H)
    xt = np.zeros((B, 2, 128, H, XR), np.float32)
    xt[:, :, :, :, PAD : PAD + H] = x.transpose(0, 1, 2, 4, 3)
    return xt.reshape(B, 2, 128, H * XR).astype(_BF16)


def make_in_maps(inputs):
    wb, wpT, bias_arr = host_prep_weights(inputs)
    xq = host_prep_x(inputs["x"])
    return [{"xp": xq[b], "wb": wb, "wp": wpT, "bias": bias_arr} for b in range(B)]


_NC_CACHE = []


def kernel(**inputs):
    from concourse import bass_utils

    if not _NC_CACHE:
        _NC_CACHE.append(build_program())
    nc = _NC_CACHE[0]
    in_maps = make_in_maps(inputs)
    res = bass_utils.run_bass_kernel_spmd(nc, in_maps, core_ids=list(range(N_CORES)))
    return np.stack(
        [r["out"].reshape(COUT, 2 * H, 2 * H) for r in res.results]
    ).astype(np.float32)


# revision 25
# speedup vs baseline: 1.0362x; 1.0111x over previous
"""ASPP + pixel-shuffle upsample + 1x1 project, on 8 TRN2 NeuronCores.

Strategy: data-parallel over batch (B=8 -> 1 image per core). Per core:
  - all convs as matmuls on the PE (bf16 inputs/weights, fp32 PSUM accum)
  - BN folded into conv weights/bias on host
  - 3x3 dilated convs = 9 shifted 1x1 taps accumulated in PSUM; each tap
    computes only its valid (non-zero-padding) region. PSUM spatial chunks
    are laid out column-major so a tap's column restriction is a contiguous
    PSUM range; x is stored row-major with 18 zero rows of top/bottom pad
    (row-shifted taps read zero rows; fully-zero chunks are skipped).
  - interleave (pixel-shuffle) is never materialized: the 1x1 projection is
    applied per-branch and its ReLU output is written with a strided AP
    directly into the interleaved position of the output row buffer
  - output rows stream back to DRAM per 16-row block
"""

import numpy as np
import ml_dtypes

B, CIN, COUT, H = 8, 256, 128, 64
PAD = 18
XR = H + 2 * PAD  # padded rows: 100
EPS = 1e-5
RATES = (6, 12, 18)
N_CORES = 8
NTAP = 28  # 1 (branch0 1x1) + 3 branches * 9 taps

_BF16 = ml_dtypes.bfloat16


def _branch_taps(t):
    """[(weight_block, sy, sx)] for branch t, center tap first."""
    if t == 0:
        return [(0, 0, 0)]
    d = RATES[t - 1]
    base = 1 + 9 * (t - 1)
    taps = []
    for ky in range(3):
        for kx in range(3):
            taps.append((base + ky * 3 + kx, (ky - 1) * d, (kx - 1) * d))
    taps.sort(key=lambda w: (w[1] != 0 or w[2] != 0))  # center first
    return taps


def build_program():
    import concourse.mybir as mybir
    import concourse.tile as tile
    from concourse import bacc

    f32, bf16 = mybir.dt.float32, mybir.dt.bfloat16
    Relu = mybir.ActivationFunctionType.Relu

    nc = bacc.Bacc("TRN2", target_bir_lowering=False, debug=False)
    xp = nc.dram_tensor("xp", [2, 128, H * XR], bf16, kind="ExternalInput")
    wb = nc.dram_tensor("wb", [2, 128, NTAP * 128], bf16, kind="ExternalInput")
    wp = nc.dram_tensor("wp", [128, 128], bf16, kind="ExternalInput")
    bias = nc.dram_tensor("bias", [128, 5], f32, kind="ExternalInput")
    out = nc.dram_tensor("out", [128, 4 * H * H], f32, kind="ExternalOutput")

    with tile.TileContext(nc) as tc:
        with (
            tc.tile_pool(name="const", bufs=1) as cpool,
            tc.tile_pool(name="bf", bufs=3) as bfpool,
            tc.tile_pool(name="ob", bufs=3) as opool,
            tc.tile_pool(name="psA", bufs=3, space="PSUM") as psA,
            tc.tile_pool(name="psB", bufs=3, space="PSUM") as psB,
        ):
            # PE warm-up: dummy matmuls on a zeroed scratch tile release the
            # HAM clock throttle while the input DMAs are still in flight
            scratch = cpool.tile([128, 512], bf16, tag="scratch")
            nc.vector.memset(scratch[:], 0.0)
            psW = psA.tile([128, 512], f32, tag="warm", bufs=1)
            for i in range(9):
                nc.tensor.matmul(
                    psW[:], lhsT=scratch[:, :128], rhs=scratch[:],
                    start=(i == 0), stop=(i == 8), skip_group_check=True,
                )
            bt = cpool.tile([128, 5], f32, tag="bias")
            nc.sync.dma_start(out=bt, in_=bias[:])
            wpt = cpool.tile([128, 128], bf16, tag="wp")
            nc.sync.dma_start(out=wpt, in_=wp[:])
            # x stored column-major: [128, 64 cols x 100 rows], rows 18..82
            # hold the image (transposed + row-padded on host), so the DMA is
            # fully contiguous and matmul rhs APs have 8 contiguous rows
            # innermost. Issue order: x chunk 0, then the weights the first
            # chunk's branches need, then x chunk 1, then branch-3 weights.
            wt = [
                cpool.tile([128, NTAP * 128], bf16, tag=f"w{c}", name=f"w{c}")
                for c in range(2)
            ]
            xtile = [
                cpool.tile([128, H * XR], bf16, tag=f"x{c}", name=f"x{c}")
                for c in range(2)
            ]
            x3t = [
                xtile[c].rearrange("p (w h) -> p w h", h=XR) for c in range(2)
            ]
            nc.sync.dma_start(out=xtile[0], in_=xp[0])
            nc.sync.dma_start(out=wt[0][:, : 19 * 128], in_=wb[0][:, : 19 * 128])
            nc.sync.dma_start(out=xtile[1], in_=xp[1])
            nc.sync.dma_start(out=wt[1][:, : 19 * 128], in_=wb[1][:, : 19 * 128])
            for c in range(2):  # branch 3 weights last
                nc.sync.dma_start(out=wt[c][:, 19 * 128 :], in_=wb[c][:, 19 * 128 :])

            out3 = out.rearrange("p (a b) -> p a b", b=2 * H)
            for k in range(8):  # 8-row input chunks -> output rows 16k..16k+16
                ob = opool.tile([128, 16 * 2 * H], f32, tag="ob")
                ob3 = ob.rearrange("p (a b) -> p a b", b=2 * H)
                # (out-col, out-row) view matching the col-major psum layout
                obt = ob3.rearrange("p a b -> p b a")
                done = set()
                # k=0: branches with long cin-chunk-0 prefixes first, so the
                # PE has work before x chunk 1 lands
                for t in ([1, 2, 3, 0] if k == 0 else range(4)):
                    ps = psA.tile([128, 512], f32, tag="ps")
                    mms = []
                    for c in range(2):  # all cin-chunk-0 taps first
                        for blk, sy, sx in _branch_taps(t):
                            if 8 * k + 8 + sy <= 0 or 8 * k + sy >= H:
                                continue  # all rows read zero-pad: contributes 0
                            c0, c1 = max(0, -sx), min(H, H - sx)
                            mms.append((blk, sy, sx, c0, c1, c))
                    n = len(mms)
                    for idx, (blk, sy, sx, c0, c1, c) in enumerate(mms):
                        r0 = PAD + 8 * k + sy
                        rhs = x3t[c][:, c0 + sx : c1 + sx, r0 : r0 + 8]
                        dst = ps[:] if (c1 - c0) == H else ps[:, c0 * 8 : c1 * 8]
                        nc.tensor.matmul(
                            dst,
                            lhsT=wt[c][:, blk * 128 : (blk + 1) * 128],
                            rhs=rhs,
                            start=(idx == 0),
                            stop=(idx == n - 1),
                        )
                    bftile = bfpool.tile([128, 512], bf16, tag="bf")
                    nc.scalar.activation(bftile[:], ps[:], Relu, bias=bt[:, t : t + 1])
                    ps2 = psB.tile([128, 512], f32, tag="ps2")
                    nc.tensor.matmul(
                        ps2[:], lhsT=wpt[:], rhs=bftile[:], start=True, stop=True
                    )
                    r_, c_ = t // 2, t % 2
                    ps2v = ps2.rearrange("p (w h) -> p w h", h=8)
                    if k < 7:
                        nc.scalar.activation(
                            obt[:, c_::2, r_::2], ps2v, Relu, bias=bt[:, 4:5]
                        )
                        done.add(t)
                        # stream each output-row parity out as soon as the
                        # two branches feeding it are done
                        if done >= {0, 1} and "even" not in done:
                            nc.sync.dma_start(
                                out=out3[:, 16 * k : 16 * (k + 1) : 2, :],
                                in_=ob3[:, 0::2, :],
                            )
                            done.add("even")
                        if done >= {2, 3} and "odd" not in done:
                            nc.sync.dma_start(
                                out=out3[:, 16 * k + 1 : 16 * (k + 1) : 2, :],
                                in_=ob3[:, 1::2, :],
                            )
                            done.add("odd")
                    else:
                        # last chunk: write row-halves so the final DMAs are
                        # contiguous and the very last one is small
                        for h in range(2):
                            nc.scalar.activation(
                                obt[:, c_::2, r_ + 8 * h : 8 + 8 * h : 2],
                                ps2v[:, :, 4 * h : 4 * (h + 1)],
                                Relu,
                                bias=bt[:, 4:5],
                            )
                        done.add(t)
                        if done >= {0, 1, 2, 3}:
                            for h in range(2):
                                nc.sync.dma_start(
                                    out=out3[:, 16 * k + 8 * h : 16 * k + 8 * (h + 1), :],
                                    in_=ob3[:, 8 * h : 8 * (h + 1), :],
                                )
    nc.compile()
    return nc


def host_prep_weights(inputs):
    f32 = np.float32
    scales, biases = [], []
    for t in ("0", "1", "2", "3", "p"):
        g = np.asarray(inputs[f"g{t}"], f32)
        b = np.asarray(inputs[f"b{t}"], f32)
        m = np.asarray(inputs[f"m{t}"], f32)
        v = np.asarray(inputs[f"v{t}"], f32)
        s = g / np.sqrt(v + EPS)
        scales.append(s)
        biases.append((b - m * s).astype(f32))
    bias_arr = np.stack(biases, axis=1).astype(f32)  # (128, 5)

    wtaps = np.zeros((NTAP, CIN, COUT), f32)  # [tap, ci, co]
    w0 = np.asarray(inputs["w0"], f32)[:, :, 0, 0] * scales[0][:, None]  # (co, ci)
    wtaps[0] = w0.T
    blk = 1
    for bi, key in enumerate(("w1", "w2", "w3")):
        w = np.asarray(inputs[key], f32) * scales[bi + 1][:, None, None, None]
        for ky in range(3):
            for kx in range(3):
                wtaps[blk] = w[:, :, ky, kx].T
                blk += 1
    wb = (
        wtaps.reshape(NTAP, 2, 128, COUT)
        .transpose(1, 2, 0, 3)
        .reshape(2, 128, NTAP * COUT)
        .astype(_BF16)
    )
    wpT = (
        (np.asarray(inputs["wp"], f32)[:, :, 0, 0] * scales[4][:, None])
        .T.astype(_BF16)
        .copy()
    )
    return wb, wpT, bias_arr


def host_prep_x(x):
    # transpose each image to (col, row) and bake the 18-row top/bottom zero
    # pad, matching the device's column-major SBUF layout exactly, so the
    # device DMA is one contiguous copy per cin-chunk
    x = np.asarray(x, np.float32).reshape(B, 2, 128, H, H)
    xt = np.zeros((B, 2, 128, H, XR), np.float32)
    xt[:, :, :, :, PAD : PAD + H] = x.transpose(0, 1, 2, 4, 3)
    return xt.reshape(B, 2, 128, H * XR).astype(_BF16)


def make_in_maps(inputs):
    wb, wpT, bias_arr = host_prep_weights(inputs)
    xq = host_prep_x(inputs["x"])
    return [{"xp": xq[b], "wb": wb, "wp": wpT, "bias": bias_arr} for b in range(B)]


_NC_CACHE = []


def kernel(**inputs):
    from concourse import bass_utils

    if not _NC_CACHE:
        _NC_CACHE.append(build_program())
    nc = _NC_CACHE[0]
    in_maps = make_in_maps(inputs)
    res = bass_utils.run_bass_kernel_spmd(nc, in_maps, core_ids=list(range(N_CORES)))
    return np.stack(
        [r["out"].reshape(COUT, 2 * H, 2 * H) for r in res.results]
    ).astype(np.float32)


# revision 28
# speedup vs baseline: 1.0463x; 1.0098x over previous
"""ASPP + pixel-shuffle upsample + 1x1 project, on 8 TRN2 NeuronCores.

Strategy: data-parallel over batch (B=8 -> 1 image per core). Per core:
  - all convs as matmuls on the PE (bf16 inputs/weights, fp32 PSUM accum)
  - BN folded into conv weights/bias on host
  - 3x3 dilated convs = 9 shifted 1x1 taps accumulated in PSUM; each tap
    computes only its valid (non-zero-padding) region. PSUM spatial chunks
    are laid out column-major so a tap's column restriction is a contiguous
    PSUM range; x is stored row-major with 18 zero rows of top/bottom pad
    (row-shifted taps read zero rows; fully-zero chunks are skipped).
  - interleave (pixel-shuffle) is never materialized: the 1x1 projection is
    applied per-branch and its ReLU output is written with a strided AP
    directly into the interleaved position of the output row buffer
  - output rows stream back to DRAM per 16-row block
"""

import numpy as np
import ml_dtypes

B, CIN, COUT, H = 8, 256, 128, 64
PAD = 18
XR = H + 2 * PAD  # padded rows: 100
EPS = 1e-5
RATES = (6, 12, 18)
N_CORES = 8
NTAP = 28  # 1 (branch0 1x1) + 3 branches * 9 taps

_BF16 = ml_dtypes.bfloat16


def _branch_taps(t):
    """[(weight_block, sy, sx)] for branch t, center tap first."""
    if t == 0:
        return [(0, 0, 0)]
    d = RATES[t - 1]
    base = 1 + 9 * (t - 1)
    taps = []
    for ky in range(3):
        for kx in range(3):
            taps.append((base + ky * 3 + kx, (ky - 1) * d, (kx - 1) * d))
    taps.sort(key=lambda w: (w[1] != 0 or w[2] != 0))  # center first
    return taps


def build_program(edge_trim=True):
    # edge_trim=False keeps every matmul's PSUM write 2-D so CoreSim can
    # check it; True additionally trims zero-pad rows at sy-edge chunks
    # (3-D PSUM writes the simulator can't shape-check — validated on HW)
    import concourse.mybir as mybir
    import concourse.tile as tile
    from concourse import bacc

    f32, bf16 = mybir.dt.float32, mybir.dt.bfloat16
    Relu = mybir.ActivationFunctionType.Relu

    nc = bacc.Bacc("TRN2", target_bir_lowering=False, debug=False)
    xp = nc.dram_tensor("xp", [2, 128, H * XR], bf16, kind="ExternalInput")
    wb = nc.dram_tensor("wb", [2, 128, NTAP * 128], bf16, kind="ExternalInput")
    wp = nc.dram_tensor("wp", [128, 128], bf16, kind="ExternalInput")
    bias = nc.dram_tensor("bias", [128, 5], f32, kind="ExternalInput")
    out = nc.dram_tensor("out", [128, 4 * H * H], f32, kind="ExternalOutput")

    with tile.TileContext(nc) as tc:
        with (
            tc.tile_pool(name="const", bufs=1) as cpool,
            tc.tile_pool(name="bf", bufs=3) as bfpool,
            tc.tile_pool(name="ob", bufs=3) as opool,
            tc.tile_pool(name="psA", bufs=3, space="PSUM") as psA,
            tc.tile_pool(name="psB", bufs=3, space="PSUM") as psB,
        ):
            # PE warm-up: dummy matmuls on a zeroed scratch tile release the
            # HAM clock throttle while the input DMAs are still in flight
            scratch = cpool.tile([128, 512], bf16, tag="scratch")
            nc.vector.memset(scratch[:], 0.0)
            psW = psA.tile([128, 512], f32, tag="warm", bufs=1)
            for i in range(20):
                nc.tensor.matmul(
                    psW[:], lhsT=scratch[:, :128], rhs=scratch[:],
                    start=(i == 0), stop=(i == 19), skip_group_check=True,
                )
            bt = cpool.tile([128, 5], f32, tag="bias")
            nc.sync.dma_start(out=bt, in_=bias[:])
            wpt = cpool.tile([128, 128], bf16, tag="wp")
            nc.sync.dma_start(out=wpt, in_=wp[:])
            # x stored column-major: [128, 64 cols x 100 rows], rows 18..82
            # hold the image (transposed + row-padded on host), so the DMA is
            # fully contiguous and matmul rhs APs have 8 contiguous rows
            # innermost. Issue order: x chunk 0, then the weights the first
            # chunk's branches need, then x chunk 1, then branch-3 weights.
            wt = [
                cpool.tile([128, NTAP * 128], bf16, tag=f"w{c}", name=f"w{c}")
                for c in range(2)
            ]
            xtile = [
                cpool.tile([128, H * XR], bf16, tag=f"x{c}", name=f"x{c}")
                for c in range(2)
            ]
            x3t = [
                xtile[c].rearrange("p (w h) -> p w h", h=XR) for c in range(2)
            ]
            nc.sync.dma_start(out=xtile[0], in_=xp[0])
            nc.sync.dma_start(out=wt[0][:, : 19 * 128], in_=wb[0][:, : 19 * 128])
            nc.sync.dma_start(out=xtile[1], in_=xp[1])
            nc.sync.dma_start(out=wt[1][:, : 19 * 128], in_=wb[1][:, : 19 * 128])
            for c in range(2):  # branch 3 weights last
                nc.sync.dma_start(out=wt[c][:, 19 * 128 :], in_=wb[c][:, 19 * 128 :])

            out3 = out.rearrange("p (a b) -> p a b", b=2 * H)
            for k in range(8):  # 8-row input chunks -> output rows 16k..16k+16
                ob = opool.tile([128, 16 * 2 * H], f32, tag="ob")
                ob3 = ob.rearrange("p (a b) -> p a b", b=2 * H)
                # (out-col, out-row) view matching the col-major psum layout
                obt = ob3.rearrange("p a b -> p b a")
                done = set()
                # k=0: branches with long cin-chunk-0 prefixes first, so the
                # PE has work before x chunk 1 lands
                for t in ([1, 2, 3, 0] if k == 0 else range(4)):
                    ps = psA.tile([128, 512], f32, tag="ps")
                    mms = []
                    for c in range(2):  # all cin-chunk-0 taps first
                        for blk, sy, sx in _branch_taps(t):
                            if 8 * k + 8 + sy <= 0 or 8 * k + sy >= H:
                                continue  # all rows read zero-pad: contributes 0
                            c0, c1 = max(0, -sx), min(H, H - sx)
                            mms.append((blk, sy, sx, c0, c1, c))
                    n = len(mms)
                    ps3 = ps.rearrange("p (w h) -> p w h", h=8)
                    for idx, (blk, sy, sx, c0, c1, c) in enumerate(mms):
                        r0 = PAD + 8 * k + sy
                        # rows of this chunk whose input row is real data
                        # (the rest read zero pad: contribute nothing)
                        a0 = max(0, -sy - 8 * k) if edge_trim else 0
                        a1 = min(8, H - sy - 8 * k) if edge_trim else 8
                        first, last = idx == 0, idx == n - 1
                        if (a0, a1) != (0, 8) and not first:
                            rhs = x3t[c][:, c0 + sx : c1 + sx, r0 + a0 : r0 + a1]
                            dst = ps3[:, c0:c1, a0:a1]
                        elif (c1 - c0) == H:
                            rhs = x3t[c][:, c0 + sx : c1 + sx, r0 : r0 + 8]
                            dst = ps[:]
                        else:
                            rhs = x3t[c][:, c0 + sx : c1 + sx, r0 : r0 + 8]
                            dst = ps[:, c0 * 8 : c1 * 8]
                        nc.tensor.matmul(
                            dst,
                            lhsT=wt[c][:, blk * 128 : (blk + 1) * 128],
                            rhs=rhs,
                            start=first,
                            stop=last,
                        )
                    bftile = bfpool.tile([128, 512], bf16, tag="bf")
                    nc.scalar.activation(bftile[:], ps[:], Relu, bias=bt[:, t : t + 1])
                    ps2 = psB.tile([128, 512], f32, tag="ps2")
                    nc.tensor.matmul(
                        ps2[:], lhsT=wpt[:], rhs=bftile[:], start=True, stop=True
                    )
                    r_, c_ = t // 2, t % 2
                    ps2v = ps2.rearrange("p (w h) -> p w h", h=8)
                    if k < 7:
                        nc.scalar.activation(
                            obt[:, c_::2, r_::2], ps2v, Relu, bias=bt[:, 4:5]
                        )
                        done.add(t)
                        # stream each output-row parity out as soon as the
                        # two branches feeding it are done
                        if done >= {0, 1} and "even" not in done:
                            nc.sync.dma_start(
                                out=out3[:, 16 * k : 16 * (k + 1) : 2, :],
                                in_=ob3[:, 0::2, :],
                            )
                            done.add("even")
                        if done >= {2, 3} and "odd" not in done:
                            nc.sync.dma_start(
                                out=out3[:, 16 * k + 1 : 16 * (k + 1) : 2, :],
                                in_=ob3[:, 1::2, :],
                            )
                            done.add("odd")
                    else:
                        # last chunk: write row-halves so the final DMAs are
                        # contiguous and the very last one is small
                        for h in range(2):
                            nc.scalar.activation(
                                obt[:, c_::2, r_ + 8 * h : 8 + 8 * h : 2],
                                ps2v[:, :, 4 * h : 4 * (h + 1)],
                                Relu,
                                bias=bt[:, 4:5],
                            )
                        done.add(t)
                        if done >= {0, 1, 2, 3}:
                            for h in range(2):
                                nc.sync.dma_start(
                                    out=out3[:, 16 * k + 8 * h : 16 * k + 8 * (h + 1), :],
                                    in_=ob3[:, 8 * h : 8 * (h + 1), :],
                                )
    nc.compile()
    return nc


def host_prep_weights(inputs):
    f32 = np.float32
    scales, biases = [], []
    for t in ("0", "1", "2", "3", "p"):
        g = np.asarray(inputs[f"g{t}"], f32)
        b = np.asarray(inputs[f"b{t}"], f32)
        m = np.asarray(inputs[f"m{t}"], f32)
        v = np.asarray(inputs[f"v{t}"], f32)
        s = g / np.sqrt(v + EPS)
        scales.append(s)
        biases.append((b - m * s).astype(f32))
    bias_arr = np.stack(biases, axis=1).astype(f32)  # (128, 5)

    wtaps = np.zeros((NTAP, CIN, COUT), f32)  # [tap, ci, co]
    w0 = np.asarray(inputs["w0"], f32)[:, :, 0, 0] * scales[0][:, None]  # (co, ci)
    wtaps[0] = w0.T
    blk = 1
    for bi, key in enumerate(("w1", "w2", "w3")):
        w = np.asarray(inputs[key], f32) * scales[bi + 1][:, None, None, None]
        for ky in range(3):
            for kx in range(3):
                wtaps[blk] = w[:, :, ky, kx].T
                blk += 1
    wb = (
        wtaps.reshape(NTAP, 2, 128, COUT)
        .transpose(1, 2, 0, 3)
        .reshape(2, 128, NTAP * COUT)
        .astype(_BF16)
    )
    wpT = (
        (np.asarray(inputs["wp"], f32)[:, :, 0, 0] * scales[4][:, None])
        .T.astype(_BF16)
        .copy()
    )
    return wb, wpT, bias_arr


def host_prep_x(x):
    # transpose each image to (col, row) and bake the 18-row top/bottom zero
    # pad, matching the device's column-major SBUF layout exactly, so the
    # device DMA is one contiguous copy per cin-chunk
    x = np.asarray(x, np.float32).reshape(B, 2, 128, H, H)
    xt = np.zeros((B, 2, 128, H, XR), np.float32)
    xt[:, :, :, :, PAD : PAD + H] = x.transpose(0, 1, 2, 4, 3)
    return xt.reshape(B, 2, 128, H * XR).astype(_BF16)


def make_in_maps(inputs):
    wb, wpT, bias_arr = host_prep_weights(inputs)
    xq = host_prep_x(inputs["x"])
    return [{"xp": xq[b], "wb": wb, "wp": wpT, "bias": bias_arr} for b in range(B)]


_NC_CACHE = []


def kernel(**inputs):
    from concourse import bass_utils

    if not _NC_CACHE:
        _NC_CACHE.append(build_program())
    nc = _NC_CACHE[0]
    in_maps = make_in_maps(inputs)
    res = bass_utils.run_bass_kernel_spmd(nc, in_maps, core_ids=list(range(N_CORES)))
    return np.stack(
        [r["out"].reshape(COUT, 2 * H, 2 * H) for r in res.results]
    ).astype(np.float32)
